# revision 1
# baseline (speedup 1.0000x reference)
"""CBOW negative-sampling loss kernel for 8 TRN2 NeuronCores.

Strategy (data-parallel, per sharding hint):
  - Shard the batch (B=16384) across 8 cores -> 2048 rows/core.
  - Replicate both embedding tables in each core's DRAM.
  - Per core: 336 indirect DMAs gather the 43008 embedding rows
    (the qPoolDynamic ucode consumes one index per partition per
    instruction, i.e. 128 rows / 64KB per ~1us GpSimd instruction;
    the batched dma_gather ucode was measured ~20 GB/s/core on this
    access pattern, so per-128-row indirect gathers are the fastest
    available primitive).  DVE computes the per-row loss terms, ACT
    the log-sigmoids; per-tile partial sums are DMA'd out and the
    final scalar reduction happens on host.
"""

import numpy as np

import concourse.bacc as bacc
import concourse.bass as bass
import concourse.mybir as mybir
import concourse.tile as tile
from concourse.bass_utils import run_bass_kernel_spmd

VOCAB = 100000
DIM = 128
B = 16384
CWIN = 10
K = 10
EPS = 1e-9
NCORES = 8
P = 128
BPC = B // NCORES            # 2048 batch rows per core
NTILES = BPC // P            # 16 tiles of 128 rows
CHUNK = 2                    # batch-tiles per gather chunk
NCHUNKS = NTILES // CHUNK
NIDX = CWIN + 1 + K          # 21 lookups per batch row

F32 = mybir.dt.float32
MULT = mybir.AluOpType.mult
ADD = mybir.AluOpType.add
AX_X = mybir.AxisListType.X
SIGMOID = mybir.ActivationFunctionType.Sigmoid
LN = mybir.ActivationFunctionType.Ln

GATHER_BUFS = 3


def build_kernel_body(tc, idx, in_emb, out_emb, usum):
    """Emit the per-core program.

    idx:    [P, NTILES*NIDX] int32 SBUF-layout indices.  Cols 0..159 are
            context lookups (free pos t*10+j -> in_emb row for batch row
            t*128+partition, window slot j); cols 160..335 are target+neg
            lookups (free pos 160 + t*11 + j -> out_emb row; j=0 target,
            j=1..10 negatives).
    usum:   [P, NTILES] f32; column t = per-row sum of
            log(sigmoid(pos)+eps) + sum_k log(sigmoid(-neg_k)+eps).
    """
    nc = tc.nc
    ctx_cols = NTILES * CWIN          # 160
    with (
        tc.tile_pool(name="io", bufs=1) as io_pool,
        tc.tile_pool(name="gather", bufs=GATHER_BUFS) as gpool,
        tc.tile_pool(name="work", bufs=2) as wpool,
    ):
        idx_t = io_pool.tile([P, NTILES * NIDX], mybir.dt.int32)
        nc.sync.dma_start(out=idx_t[:], in_=idx[:, :])

        eps_t = io_pool.tile([P, 1], F32)
        nc.vector.memset(eps_t[:], EPS)

        us = io_pool.tile([P, NTILES], F32)

        for c in range(NCHUNKS):
            ctx_g = gpool.tile([P, CHUNK * CWIN * DIM], F32, tag="ctx")
            tn_g = gpool.tile([P, CHUNK * (K + 1) * DIM], F32, tag="tn")
            c0 = c * CHUNK * CWIN
            t0 = ctx_cols + c * CHUNK * (K + 1)
            # one 128-row gather per (tile, lookup) column
            for q in range(CHUNK * CWIN):
                nc.gpsimd.indirect_dma_start(
                    out=ctx_g[:, q * DIM : (q + 1) * DIM],
                    out_offset=None,
                    in_=in_emb[:, :],
                    in_offset=bass.IndirectOffsetOnAxis(
                        ap=idx_t[:, c0 + q : c0 + q + 1], axis=0
                    ),
                )
            for q in range(CHUNK * (K + 1)):
                nc.gpsimd.indirect_dma_start(
                    out=tn_g[:, q * DIM : (q + 1) * DIM],
                    out_offset=None,
                    in_=out_emb[:, :],
                    in_offset=bass.IndirectOffsetOnAxis(
                        ap=idx_t[:, t0 + q : t0 + q + 1], axis=0
                    ),
                )

            for b in range(CHUNK):
                t_idx = c * CHUNK + b
                bc = b * CWIN * DIM        # base into ctx_g
                bt = b * (K + 1) * DIM     # base into tn_g

                # context sum over the 10 window rows (tree of adds)
                a1 = wpool.tile([P, 5 * DIM], F32, tag="a1")
                nc.vector.tensor_add(
                    a1[:], ctx_g[:, bc : bc + 5 * DIM],
                    ctx_g[:, bc + 5 * DIM : bc + 10 * DIM],
                )
                b1 = wpool.tile([P, 2 * DIM], F32, tag="b1")
                nc.vector.tensor_add(
                    b1[:], a1[:, 0 : 2 * DIM], a1[:, 2 * DIM : 4 * DIM]
                )
                csum = wpool.tile([P, DIM], F32, tag="csum")
                nc.vector.tensor_add(csum[:], b1[:, 0:DIM], b1[:, DIM : 2 * DIM])
                nc.vector.tensor_add(csum[:], csum[:], a1[:, 4 * DIM : 5 * DIM])

                # scores: s[:,0] = sum_d csum*tgt ; s[:,1+k] = sum_d csum*neg_k
                s = wpool.tile([P, 1 + K], F32, tag="s")
                prod = wpool.tile([P, (K + 1) * DIM], F32, tag="prod")
                prod3 = prod[:].rearrange("p (k d) -> p k d", d=DIM)
                tn3 = tn_g[:, bt : bt + (K + 1) * DIM].rearrange(
                    "p (k d) -> p k d", d=DIM
                )
                csum_b = csum[:][:, None, :].to_broadcast([P, K + 1, DIM])
                nc.vector.tensor_tensor(prod3, tn3, csum_b, MULT)
                nc.vector.tensor_reduce(
                    out=s[:, 0 : 1 + K], in_=prod3, axis=AX_X, op=ADD
                )
                # flip the target column so sigmoid(-0.1*s) = sigmoid(+pos)
                nc.vector.tensor_scalar_mul(s[:, 0:1], s[:, 0:1], -1.0)

                # loss terms; the /10 context-mean is folded into the
                # activation scale
                sig = wpool.tile([P, 1 + K], F32, tag="sig")
                nc.scalar.activation(sig[:], s[:], SIGMOID, scale=-0.1)
                lnv = wpool.tile([P, 1 + K], F32, tag="lnv")
                nc.scalar.activation(
                    lnv[:], sig[:], LN, bias=eps_t[:],
                    accum_out=us[:, t_idx : t_idx + 1],
                )

        nc.sync.dma_start(out=usum[:, :], in_=us[:])


def build_nc():
    nc = bacc.Bacc(
        "TRN2",
        target_bir_lowering=False,
        debug=False,
        enable_asserts=False,
        num_devices=NCORES,
    )
    idx = nc.dram_tensor(
        "idx", [P, NTILES * NIDX], mybir.dt.int32, kind="ExternalInput"
    )
    in_emb = nc.dram_tensor("in_emb", [VOCAB, DIM], F32, kind="ExternalInput")
    out_emb = nc.dram_tensor("out_emb", [VOCAB, DIM], F32, kind="ExternalInput")
    usum = nc.dram_tensor("usum", [P, NTILES], F32, kind="ExternalOutput")
    with tile.TileContext(nc) as tc:
        build_kernel_body(tc, idx.ap(), in_emb.ap(), out_emb.ap(), usum.ap())
    nc.compile()
    return nc


def _wrap16(arr):
    """[n] int16 -> [128, n/16] SBUF layout for dma_gather index lists
    (kept for the experiment scripts)."""
    w = arr.reshape(-1, 16).T
    return np.tile(w, (8, 1))


def make_in_maps(context, target, negatives, in_emb, out_emb):
    context = np.asarray(context).astype(np.int32)
    target = np.asarray(target).astype(np.int32)
    negatives = np.asarray(negatives).astype(np.int32)
    in_emb = np.ascontiguousarray(np.asarray(in_emb, dtype=np.float32))
    out_emb = np.ascontiguousarray(np.asarray(out_emb, dtype=np.float32))
    tn_full = np.concatenate([target[:, None], negatives], axis=1)  # [B, 11]
    in_maps = []
    for c in range(NCORES):
        ctx_sl = context[c * BPC : (c + 1) * BPC]  # [2048, 10]
        tn_sl = tn_full[c * BPC : (c + 1) * BPC]   # [2048, 11]
        ctx_tiles = (
            ctx_sl.reshape(NTILES, P, CWIN)
            .transpose(1, 0, 2)
            .reshape(P, NTILES * CWIN)
        )
        tn_tiles = (
            tn_sl.reshape(NTILES, P, K + 1)
            .transpose(1, 0, 2)
            .reshape(P, NTILES * (K + 1))
        )
        tiles = np.concatenate([ctx_tiles, tn_tiles], axis=1)  # [P, 336]
        in_maps.append(
            {
                "idx": np.ascontiguousarray(tiles),
                "in_emb": in_emb,
                "out_emb": out_emb,
            }
        )
    return in_maps


_NC_CACHE = []
LAST_RESULT = None  # BassKernelResults of the most recent run (for profiling)


def kernel(**inputs) -> np.ndarray:
    global LAST_RESULT
    in_maps = make_in_maps(
        inputs["context"],
        inputs["target"],
        inputs["negatives"],
        inputs["in_emb"],
        inputs["out_emb"],
    )
    if not _NC_CACHE:
        _NC_CACHE.append(build_nc())
    nc = _NC_CACHE[0]
    res = run_bass_kernel_spmd(nc, in_maps, core_ids=list(range(NCORES)))
    LAST_RESULT = res
    total = sum(float(r["usum"].astype(np.float64).sum()) for r in res.results)
    return np.array(-total / B, dtype=np.float32)



# revision 17
# speedup vs baseline: 1.0697x; 1.0697x over previous
"""CBOW negative-sampling loss kernel for 8 TRN2 NeuronCores.

Strategy (data-parallel, per sharding hint):
  - Shard the batch (B=16384) across 8 cores -> 2048 rows/core,
    replicate both embedding tables in each core's DRAM.
  - The old kernel issued one 128-row indirect DMA per (tile, lookup)
    column: 336 instructions/core, each paying the ~994ns SWDGE
    descriptor-generation fixed cost on the Pool engine -> ~350us of
    serialized DGE.  This version batches 2-tile chunks into one
    indirect DMA (10-22 indices per partition, 1280-2816 descriptors),
    so the fixed cost is paid 24 times instead of 336 and the kernel
    runs at the DMA transfer roofline (~62us/core for the 22MB of
    gathered rows).
  - The DMA engines also help with the math: the even-j context rows
    are gathered plainly, then the odd-j rows are gathered with
    compute_op=add into the same SBUF tile (walrus supports cce add,
    not mult), so DVE only folds 5 partial rows per tile instead of 10.
  - Per chunk the DMA stream is [ctx-even, tgt+neg, ctx-odd] so the
    score pipeline (DVE multiply by broadcast csum in-place, then
    per-(tile,k) reduce) streams ~1 chunk behind the transfers; engine
    queues never head-of-line-block a DMA dependency, and every chunk's
    tiles stay resident in SBUF (~140KB of 208KB per partition).
  - -log(sigmoid(+-score/10)) is computed as ln(1+exp(-+score/10)):
    Exp and Ln share one activation-function table set (preloaded
    explicitly), so no table reload sits on the critical path; the sign
    split lives in two strided Exp calls per half (target columns get
    scale -0.1, negative columns +0.1).  The ln accumulates per
    partition; per-core partials are summed on host.
"""

import numpy as np

import concourse.bacc as bacc
import concourse.bass as bass
import concourse.mybir as mybir
import concourse.tile as tile
from concourse.bass_utils import run_bass_kernel_spmd

VOCAB = 100000
DIM = 128
B = 16384
CWIN = 10
K = 10
EPS = 1e-9
NCORES = 8
P = 128
BPC = B // NCORES            # 2048 batch rows per core
NTILES = BPC // P            # 16 tiles of 128 rows
# batch-tiles per gather chunk; the last two are single tiles so the
# final reduces on the critical path are as short as possible
CHUNK_SIZES = [2] * 7 + [1] * 2
NCHUNKS = len(CHUNK_SIZES)
CHUNK_T0 = [sum(CHUNK_SIZES[:i]) for i in range(NCHUNKS)]  # first tile of chunk
# the last two chunks skip the even/odd DMA-add pairing: a single plain
# gather, summed by a full DVE add tree
PLAIN_C = NCHUNKS - 2
NIDX = CWIN + 1 + K          # 21 lookups per batch row
CTX_COLS = NTILES * CWIN     # 160
KP1 = K + 1
# chunks up to SPLIT_C-1 feed the early exp/ln half, the rest the late one
SPLIT_C = 7
SPLIT_COL = CHUNK_T0[SPLIT_C] * KP1
# the very last tile's tn gather + reduce are split k=[0,6) / [6,11) so the
# final reduce only waits on a 5-row-per-partition transfer
LKA = 6

F32 = mybir.dt.float32
ADD = mybir.AluOpType.add
MULT = mybir.AluOpType.mult
BYP = mybir.AluOpType.bypass
AX_X = mybir.AxisListType.X
EXP = mybir.ActivationFunctionType.Exp
LN = mybir.ActivationFunctionType.Ln


def build_kernel_body(tc, idx, in_emb, out_emb, usum):
    """Emit the per-core program.

    idx: [P, 336] int32.  Per chunk c of 2 tiles:
      cols [20c, 20c+10): in_emb lookups for even window slots
        j=0,2,..,8 in (tile, i) order -> pair slot i covers j=2i;
      cols [20c+10, 20c+20): odd slots j=1,3,..,9, same order.
      cols [160+22c, 160+22c+22): out_emb lookups, (tile, k) order with
        k=0 the target and k=1..10 the negatives.
    usum: [P, 2] f32, per-partition sums over the early/late score
      columns of ln(1 + exp(-+score/10)) = -log(sigmoid(+-score/10)).
    """
    nc = tc.nc
    with (
        tc.tile_pool(name="io", bufs=1) as io_pool,
        tc.tile_pool(name="ch", bufs=1) as ch_pool,
        tc.tile_pool(name="prod", bufs=1) as pr_pool,
        tc.tile_pool(name="csum", bufs=1) as cs_pool,
        tc.tile_pool(name="work", bufs=2) as wpool,
    ):
        idx_t = io_pool.tile([P, NTILES * NIDX], mybir.dt.int32)
        nc.sync.dma_start(out=idx_t[:], in_=idx[:, :])

        one_t = io_pool.tile([P, 1], F32)
        nc.vector.memset(one_t[:], 1.0)

        s_all = io_pool.tile([P, NTILES * KP1], F32)

        chs = [ch_pool.tile(
                   [P, cs_ * (5 if i < PLAIN_C else 10) * DIM], F32,
                   tag=f"ch{i}", name=f"ch{i}")
               for i, cs_ in enumerate(CHUNK_SIZES)]
        tngs = [pr_pool.tile([P, cs_ * KP1 * DIM], F32, tag=f"pr{i}", name=f"pr{i}")
                for i, cs_ in enumerate(CHUNK_SIZES[:-1])]
        tngs.append(pr_pool.tile([P, LKA * DIM], F32, tag="pra", name="pr_a"))
        tng_b = pr_pool.tile([P, (KP1 - LKA) * DIM], F32, tag="prb", name="pr_b")
        csums = [cs_pool.tile([P, cs_ * DIM], F32, tag=f"cs{i}", name=f"cs{i}")
                 for i, cs_ in enumerate(CHUNK_SIZES)]

        # preload the one activation-table set that covers Exp, Ln and Copy
        # so the table-load pass never thrashes on the critical path
        from concourse.hw_specs import get_activation_tables
        tabs = list(get_activation_tables(nc.m.arch))
        nc.scalar.add_instruction(
            mybir.InstLoadActFuncSet(
                name=nc.get_next_instruction_name(),
                act_func_set_id=tabs.index("natural_log_exp_and_others"),
                ins=[],
                outs=[],
            )
        )

        # Phase 1: gathers, software-pipelined as [ev_i, tn_(i-1), od_i]
        # blocks: the odd ctx gather (cce add, pairing window slot 2i with
        # 2i+1 via RMW) waits on its even's completion, and the previous
        # chunk's tn transfer covers that wait so the DMA queue never
        # starves; csum_i is ready one block before tn_i lands.  The tail
        # chunks use plain full ctx gathers (no RMW chain) and their small
        # tn gathers close the stream.
        def ev_dma(c):
            ev_n = CHUNK_SIZES[c] * 5
            c0 = CHUNK_T0[c] * CWIN
            nc.gpsimd.indirect_dma_start(
                out=chs[c][:],
                out_offset=None,
                in_=in_emb[:, :],
                in_offset=bass.IndirectOffsetOnAxis(
                    ap=idx_t[:, c0 : c0 + ev_n], axis=0
                ),
            )

        def od_dma(c):
            ev_n = CHUNK_SIZES[c] * 5
            c0 = CHUNK_T0[c] * CWIN
            nc.gpsimd.indirect_dma_start(
                out=chs[c][:],
                out_offset=None,
                in_=in_emb[:, :],
                in_offset=bass.IndirectOffsetOnAxis(
                    ap=idx_t[:, c0 + ev_n : c0 + 2 * ev_n], axis=0
                ),
                compute_op=ADD,
            )

        def full_ctx_dma(c):
            c0 = CHUNK_T0[c] * CWIN
            n = CHUNK_SIZES[c] * CWIN
            nc.gpsimd.indirect_dma_start(
                out=chs[c][:],
                out_offset=None,
                in_=in_emb[:, :],
                in_offset=bass.IndirectOffsetOnAxis(
                    ap=idx_t[:, c0 : c0 + n], axis=0
                ),
            )

        def tn_dma(c, lo, hi, buf):
            t0 = CTX_COLS + CHUNK_T0[c] * KP1
            nc.gpsimd.indirect_dma_start(
                out=buf[:],
                out_offset=None,
                in_=out_emb[:, :],
                in_offset=bass.IndirectOffsetOnAxis(
                    ap=idx_t[:, t0 + lo : t0 + hi], axis=0
                ),
            )

        ev_dma(0)
        for c in range(1, PLAIN_C):
            ev_dma(c)
            tn_dma(c - 1, 0, CHUNK_SIZES[c - 1] * KP1, tngs[c - 1])
            od_dma(c - 1)
        full_ctx_dma(PLAIN_C)
        tn_dma(PLAIN_C - 1, 0, CHUNK_SIZES[PLAIN_C - 1] * KP1, tngs[PLAIN_C - 1])
        od_dma(PLAIN_C - 1)
        full_ctx_dma(PLAIN_C + 1)
        tn_dma(PLAIN_C, 0, KP1, tngs[PLAIN_C])
        tn_dma(PLAIN_C + 1, 0, LKA, tngs[PLAIN_C + 1])
        tn_dma(PLAIN_C + 1, LKA, KP1, tng_b)

        # Phase 2: fold pair-sums into csum, multiply the gathered
        # target/negative rows by broadcast csum in place, and reduce.
        # Emitted chunk-major so DVE streams ~1 chunk behind the DMAs.
        sg = io_pool.tile([P, NTILES * KP1], F32)
        lnv = io_pool.tile([P, NTILES * KP1], F32)
        us = io_pool.tile([P, 2], F32)

        def stt_add(out, a, b):
            nc.vector.tensor_tensor(out, a, b, ADD)

        def stt_mul(out, a, b):
            nc.vector.tensor_tensor(out, a, b, MULT)

        def score_tail(c, tile_ap, k0, kn, col):
            nt = CHUNK_SIZES[c]
            t4 = tile_ap.rearrange("p (t k d) -> p t k d", k=kn, d=DIM)
            cs3 = csums[c][:].rearrange("p (t d) -> p t d", d=DIM)
            cs_b = cs3[:, :, None, :].to_broadcast([P, nt, kn, DIM])
            stt_mul(t4, t4, cs_b)
            nc.vector.tensor_reduce(
                out=s_all[:, col : col + nt * kn], in_=t4, axis=AX_X, op=ADD
            )

        def exp_half(lo, hi, slot):
            # x=+s/10 for the target column, -s/10 for negatives; the loss
            # term is ln(1+exp(-x)) either way
            sv = s_all[:, lo:hi].rearrange("p (t k) -> p t k", k=KP1)
            gv = sg[:, lo:hi].rearrange("p (t k) -> p t k", k=KP1)
            nc.scalar.activation(gv[:, :, 0:1], sv[:, :, 0:1], EXP, scale=-0.1)
            nc.scalar.activation(gv[:, :, 1:KP1], sv[:, :, 1:KP1], EXP, scale=0.1)
            nc.scalar.activation(
                lnv[:, lo:hi], sg[:, lo:hi], LN,
                bias=one_t[:], accum_out=us[:, slot : slot + 1],
            )

        for c in range(NCHUNKS):
            nt = CHUNK_SIZES[c]
            cs3 = csums[c][:].rearrange("p (t d) -> p t d", d=DIM)
            if c < PLAIN_C:
                ch4 = chs[c][:].rearrange("p (t i d) -> p t i d", i=5, d=DIM)
                b2 = wpool.tile([P, nt * 2 * DIM], F32, tag="b2", name=f"b2_{c}")
                b24 = b2[:].rearrange("p (t i d) -> p t i d", i=2, d=DIM)
                stt_add(b24, ch4[:, :, 0:2, :], ch4[:, :, 2:4, :])
                stt_add(cs3, b24[:, :, 0, :], b24[:, :, 1, :])
                stt_add(cs3, cs3, ch4[:, :, 4, :])
            else:
                ch4 = chs[c][:].rearrange("p (t i d) -> p t i d", i=10, d=DIM)
                a1 = wpool.tile([P, nt * 5 * DIM], F32, tag="a1", name=f"a1_{c}")
                a14 = a1[:].rearrange("p (t i d) -> p t i d", i=5, d=DIM)
                stt_add(a14, ch4[:, :, 0:5, :], ch4[:, :, 5:10, :])
                b2 = wpool.tile([P, nt * 2 * DIM], F32, tag="b2", name=f"b2_{c}")
                b24 = b2[:].rearrange("p (t i d) -> p t i d", i=2, d=DIM)
                stt_add(b24, a14[:, :, 0:2, :], a14[:, :, 2:4, :])
                stt_add(cs3, b24[:, :, 0, :], b24[:, :, 1, :])
                stt_add(cs3, cs3, a14[:, :, 4, :])

            col = CHUNK_T0[c] * KP1
            if c < NCHUNKS - 1:
                score_tail(c, tngs[c][:], 0, KP1, col)
            else:
                # k-split so the final reduce only waits on the small
                # trailing transfer
                sv = s_all[:, col : col + KP1].rearrange(
                    "p (t k) -> p t k", k=KP1
                )
                cs3l = csums[c][:].rearrange("p (t d) -> p t d", d=DIM)
                ta = tngs[c][:].rearrange("p (t k d) -> p t k d", t=1, d=DIM)
                cs_ba = cs3l[:, :, None, :].to_broadcast([P, 1, LKA, DIM])
                stt_mul(ta, ta, cs_ba)
                nc.vector.tensor_reduce(
                    out=s_all[:, col : col + LKA], in_=ta, axis=AX_X, op=ADD
                )
                tb = tng_b[:].rearrange("p (t k d) -> p t k d", t=1, d=DIM)
                cs_bb = cs3l[:, :, None, :].to_broadcast(
                    [P, 1, KP1 - LKA, DIM]
                )
                stt_mul(tb, tb, cs_bb)
                nc.vector.tensor_reduce(
                    out=s_all[:, col + LKA : col + KP1], in_=tb,
                    axis=AX_X, op=ADD,
                )
            if c == SPLIT_C - 1:
                exp_half(0, SPLIT_COL, 0)
        exp_half(SPLIT_COL, NTILES * KP1, 1)
        nc.sync.dma_start(out=usum[:, :], in_=us[:])

def build_nc():
    nc = bacc.Bacc(
        "TRN2",
        target_bir_lowering=False,
        debug=False,
        enable_asserts=False,
        num_devices=NCORES,
    )
    idx = nc.dram_tensor(
        "idx", [P, NTILES * NIDX], mybir.dt.int32, kind="ExternalInput"
    )
    in_emb = nc.dram_tensor("in_emb", [VOCAB, DIM], F32, kind="ExternalInput")
    out_emb = nc.dram_tensor("out_emb", [VOCAB, DIM], F32, kind="ExternalInput")
    usum = nc.dram_tensor("usum", [P, 2], F32, kind="ExternalOutput")
    with tile.TileContext(nc) as tc:
        build_kernel_body(tc, idx.ap(), in_emb.ap(), out_emb.ap(), usum.ap())
    nc.compile()
    return nc


def make_in_maps(context, target, negatives, in_emb, out_emb):
    context = np.asarray(context).astype(np.int32)
    target = np.asarray(target).astype(np.int32)
    negatives = np.asarray(negatives).astype(np.int32)
    in_emb = np.ascontiguousarray(np.asarray(in_emb, dtype=np.float32))
    out_emb = np.ascontiguousarray(np.asarray(out_emb, dtype=np.float32))
    tn_full = np.concatenate([target[:, None], negatives], axis=1)  # [B, 11]
    in_maps = []
    for cid in range(NCORES):
        ctx_sl = context[cid * BPC : (cid + 1) * BPC]  # [2048, 10]
        tn_sl = tn_full[cid * BPC : (cid + 1) * BPC]   # [2048, 11]
        ctx_t = ctx_sl.reshape(NTILES, P, CWIN)        # (t, p, j)
        tn_t = tn_sl.reshape(NTILES, P, KP1)           # (t, p, k)
        ctx_cols = []
        tn_cols = []
        for c in range(NCHUNKS):
            t0, nt = CHUNK_T0[c], CHUNK_SIZES[c]
            blk = ctx_t[t0 : t0 + nt]                  # [nt, 128, 10]
            ev = blk[:, :, 0::2].transpose(1, 0, 2).reshape(P, nt * 5)
            od = blk[:, :, 1::2].transpose(1, 0, 2).reshape(P, nt * 5)
            ctx_cols += [ev, od]
            tblk = tn_t[t0 : t0 + nt]                  # [nt, 128, 11]
            tn_cols.append(tblk.transpose(1, 0, 2).reshape(P, nt * KP1))
        idx_arr = np.concatenate(ctx_cols + tn_cols, axis=1)  # [P, 336]
        in_maps.append(
            {
                "idx": np.ascontiguousarray(idx_arr),
                "in_emb": in_emb,
                "out_emb": out_emb,
            }
        )
    return in_maps


_NC_CACHE = []
LAST_RESULT = None  # BassKernelResults of the most recent run (for profiling)


def kernel(**inputs) -> np.ndarray:
    global LAST_RESULT
    in_maps = make_in_maps(
        inputs["context"],
        inputs["target"],
        inputs["negatives"],
        inputs["in_emb"],
        inputs["out_emb"],
    )
    if not _NC_CACHE:
        _NC_CACHE.append(build_nc())
    nc = _NC_CACHE[0]
    res = run_bass_kernel_spmd(nc, in_maps, core_ids=list(range(NCORES)))
    LAST_RESULT = res
    total = sum(float(r["usum"].astype(np.float64).sum()) for r in res.results)
    return np.array(total / B, dtype=np.float32)
